# revision 1
# baseline (speedup 1.0000x reference)
"""CON_GATLayer Trainium2 kernel: 8-core row-sharded GAT with dual-branch
score gathering implemented via per-partition local_scatter passes.

Self-contained: host preprocessing (index scatter-schedules, weight
augmentation) + Bass/Tile kernel builder + SPMD runner.
"""
import math
import numpy as np

import concourse.bass as bass
import concourse.tile as tile
from concourse import bacc, mybir, masks
from concourse.vector_clock import ScopedClock
from concourse.bass_utils import run_bass_kernel_spmd

f32 = mybir.dt.float32
f32r = mybir.dt.float32r
f16 = mybir.dt.float16
i16 = mybir.dt.int16
AF = mybir.ActivationFunctionType
ALU = mybir.AluOpType


class TC(tile.TileContext):
    """TileContext whose final drain splits sem waits into single-wait nops
    (walrus CoreV3 drain codegen rejects >2 wait commands per instruction)."""

    def _drain_and_barrier(self, tick_clock, wait_clock):
        nc = self.nc
        carrier = nc.sync.nop()
        wait_clock.add_sem_waits(
            carrier.ins, ScopedClock({None: tick_clock.global_clock})
        )
        si = carrier.ins.sync_info
        waits = list(si.on_wait) if si and si.on_wait else []
        if len(waits) > 2:
            si.on_wait = []
            for w in waits:
                nop = nc.sync.nop()
                nsi = nop.ins.sync_info
                if nsi is None:
                    nop.ins.sync_info = mybir.SyncInfo(on_wait=[w], on_update=[])
                else:
                    nsi.on_wait = [w]
        nc.sync.drain()
        nc.all_engine_barrier()
        assert self.sems is not None
        popped = nc._tile_sem_poison_stack.pop()
        assert popped is self._sem_poison
        nc.clear_and_free_semaphores(list(self.sems.allocated().values()))
        nc.all_engine_barrier()


# ---------------------------------------------------------------------------
# configuration
# ---------------------------------------------------------------------------

class Cfg:
    def __init__(self, N=3072, IN=256, DH=64, DV=32, H=4, ncores=8):
        P = 128
        self.N, self.IN, self.DH, self.DV, self.H, self.ncores = N, IN, DH, DV, H, ncores
        self.P = P
        assert N % (ncores * P) == 0
        self.RPC = N // ncores          # rows per core
        self.NT = self.RPC // P         # 128-row tiles per core
        self.HALF = N // 2              # scatter dst width
        assert self.HALF * 32 < 2**16, "local_scatter num_elems limit"
        # tail pass schedule (lengths of T-prefix re-reads in the stream)
        TW = N // 8 + 64
        passes = [TW, TW // 2, TW // 4, 32, 16, 8, 8, 8, 8, 8]
        self.TW = TW
        self.passes = passes
        offs = []
        o = N
        for L in passes:
            offs.append(o)
            o += L
        self.passoff = offs             # column offset of each pass region
        self.SLEN = o                   # total stream length
        assert self.SLEN % 2 == 0
        # support(p): how many targets a source at T-position p can serve
        self.support = [1 + sum(1 for L in passes if L > p) for p in range(TW)]
        # matmul chunking
        self.FCH = min(512, N)          # att matmul moving free chunk
        self.PIECE = min(1024, N)       # att psum piece width
        assert N % self.PIECE == 0 and self.PIECE % self.FCH == 0
        self.kchunks = []
        o = 0
        while o < IN:
            c = min(P, IN - o)
            self.kchunks.append((o, c))
            o += c
        self.kchunks.append((IN, 1))    # bias row
        self.INA = IN + 1
        self.VG = DV + 1                # v-group width (v columns + ones)
        self.NJ = N // P                # number of j chunks
        self.HPT = 2 if H >= 2 else 1   # heads per kt/qt tile
        self.NHP = H // self.HPT


# ---------------------------------------------------------------------------
# host preprocessing
# ---------------------------------------------------------------------------

def build_scatter_indices(cfg, idx, edge):
    """idx: (N,N) int source-position matrix (gather: G[i,j]=src[i, idx[i,j]]).
    edge: (N,N) 0/1 — only edge positions need correct G.
    Returns (sA, sB, cidx): int16 arrays (N, SLEN), (N, SLEN), (N, N).
    sA/sB are per-half scatter idx streams; cidx the tail-compact idx."""
    N, HALF, TW = cfg.N, cfg.HALF, cfg.TW
    passes, passoff = cfg.passes, cfg.passoff
    ii, jj = np.nonzero(edge)
    mm = idx[ii, jj].astype(np.int64)
    key = ii.astype(np.int64) * N + mm
    order = np.argsort(key, kind="stable")
    ii, jj, mm, key = ii[order], jj[order], mm[order], key[order]
    first = np.r_[True, key[1:] != key[:-1]]
    grp_start = np.maximum.accumulate(np.where(first, np.arange(len(key)), 0))
    rank = np.arange(len(key)) - grp_start
    # counts per key
    uk, inv, cnt = np.unique(key, return_inverse=True, return_counts=True)
    # tail keys: count >= 2
    tail_mask_k = cnt >= 2
    tk = uk[tail_mask_k]
    tcnt = cnt[tail_mask_k]
    t_i = tk // N
    # position within row, sorted by count desc (stable)
    t_order = np.lexsort((-tcnt, t_i))
    tk_s, tcnt_s, ti_s = tk[t_order], tcnt[t_order], t_i[t_order]
    firstt = np.r_[True, ti_s[1:] != ti_s[:-1]]
    gs = np.maximum.accumulate(np.where(firstt, np.arange(len(ti_s)), 0))
    tpos_s = np.arange(len(ti_s)) - gs
    if len(tpos_s):
        assert tpos_s.max() < TW, f"tail buffer overflow: {tpos_s.max()} >= {TW}"
        sup = np.array(cfg.support)[tpos_s]
        assert np.all(tcnt_s - 1 <= sup), "tail pass schedule insufficient"
    # map key -> tpos
    tpos_of = np.full(len(uk), -1, np.int64)
    tpos_of[np.searchsorted(uk, tk_s)] = tpos_s
    el_tpos = tpos_of[inv]  # per element

    sA = np.full((N, cfg.SLEN), -1, np.int16)
    sB = np.full((N, cfg.SLEN), -1, np.int16)
    cidx = np.full((N, N), -1, np.int16)
    cidx[ti_s, (tk_s % N)] = tpos_s.astype(np.int16)

    half = jj >= HALF
    jloc = np.where(half, jj - HALF, jj).astype(np.int16)

    r0 = rank == 0
    dst = np.where(half, 1, 0)
    # rank 0 -> main region, column = m
    selA = r0 & (dst == 0)
    selB = r0 & (dst == 1)
    sA[ii[selA], mm[selA]] = jloc[selA]
    sB[ii[selB], mm[selB]] = jloc[selB]
    # rank >= 1 -> tail pass (rank-1), column = passoff[r-1] + tpos
    rt = rank >= 1
    pr = rank[rt] - 1
    assert len(pr) == 0 or pr.max() < len(passes), "too many copies needed"
    col = np.array(passoff)[pr] + el_tpos[rt]
    selA = np.zeros(len(ii), bool)
    selA[np.nonzero(rt)[0]] = ~half[rt]
    selB = np.zeros(len(ii), bool)
    selB[np.nonzero(rt)[0]] = half[rt]
    colA = col[~half[rt]]
    colB = col[half[rt]]
    sA[ii[selA], colA] = jloc[selA]
    sB[ii[selB], colB] = jloc[selB]
    return sA, sB, cidx


def host_prep(cfg, inputs):
    """Returns per-core input maps (list of dicts) and metadata."""
    N, IN, DH, DV, H = cfg.N, cfg.IN, cfg.DH, cfg.DV, cfg.H
    x = np.asarray(inputs["x"], np.float32)
    fst = np.asarray(inputs["fst_graph"], np.float32)
    sec = np.asarray(inputs["sec_graph"], np.float32)
    n2c = np.asarray(inputs["n2c"]).astype(np.int32)
    c2n = np.asarray(inputs["c2n"]).astype(np.int32)

    scale = 1.0 / math.sqrt(DH)
    xTa = np.empty((IN + 1, N), np.float32)
    xTa[:IN] = x.T
    xTa[IN] = 1.0

    def aug(W, b, s=1.0):
        Wa = np.empty((IN + 1, W.shape[1]), np.float32)
        Wa[:IN] = np.asarray(W, np.float32) * s
        Wa[IN] = np.asarray(b, np.float32) * s
        return Wa

    wq1 = aug(inputs["Wq1"], inputs["bq1"], scale)
    wk1 = aug(inputs["Wk1"], inputs["bk1"])
    wq2 = aug(inputs["Wq2"], inputs["bq2"], scale)
    wk2 = aug(inputs["Wk2"], inputs["bk2"])
    # v' layout: per (branch b, head h) group of (DV+1) cols: [Wv_h | ones]
    VG = cfg.VG
    wva = np.zeros((IN + 1, 2 * H * VG), np.float32)
    for b, (Wv, bv) in enumerate(
        [(inputs["Wv1"], inputs["bv1"]), (inputs["Wv2"], inputs["bv2"])]
    ):
        Wv = np.asarray(Wv, np.float32)
        bv = np.asarray(bv, np.float32)
        for h in range(H):
            g = b * H + h
            wva[:IN, g * VG : g * VG + DV] = Wv[:, h * DV : (h + 1) * DV]
            wva[IN, g * VG : g * VG + DV] = bv[h * DV : (h + 1) * DV]
            wva[IN, g * VG + DV] = 1.0

    # branch 1 gathers att2 with c2n, edge1 = fst.T; branch 2 gathers att1
    # with n2c, edge2 = sec.T
    e1 = (fst.T != 0.0)
    e2 = (sec.T != 0.0)
    s1a, s1b, c1 = build_scatter_indices(cfg, c2n, e1)
    s2a, s2b, c2 = build_scatter_indices(cfg, n2c, e2)

    maps = []
    for c in range(cfg.ncores):
        r0, r1 = c * cfg.RPC, (c + 1) * cfg.RPC
        maps.append(dict(
            xta=xTa, wq1=wq1, wk1=wk1, wq2=wq2, wk2=wk2, wva=wva,
            s1a=np.ascontiguousarray(s1a[r0:r1]),
            s1b=np.ascontiguousarray(s1b[r0:r1]),
            s2a=np.ascontiguousarray(s2a[r0:r1]),
            s2b=np.ascontiguousarray(s2b[r0:r1]),
            c1=np.ascontiguousarray(c1[r0:r1]),
            c2=np.ascontiguousarray(c2[r0:r1]),
        ))
    return maps

# ---------------------------------------------------------------------------
# device kernel builder
# ---------------------------------------------------------------------------

def build_module(cfg, reps=1, skip=()):
    import os
    N, IN, DH, DV, H, P = cfg.N, cfg.IN, cfg.DH, cfg.DV, cfg.H, cfg.P
    NT, HALF, TW, SLEN = cfg.NT, cfg.HALF, cfg.TW, cfg.SLEN
    VG, NJ, INA, RPC = cfg.VG, cfg.NJ, cfg.INA, cfg.RPC
    nc = bacc.Bacc("TRN2", target_bir_lowering=False, debug=False,
                   num_devices=cfg.ncores)

    def dram_in(name, shape, dt):
        return nc.dram_tensor(name, list(shape), dt, kind="ExternalInput").ap()

    xta = dram_in("xta", (INA, N), f32)
    xtq = dram_in("xtq", (INA, RPC), f32)
    wq = [dram_in("wq1", (INA, H * DH), f32), dram_in("wq2", (INA, H * DH), f32)]
    wk = [dram_in("wk1", (INA, H * DH), f32), dram_in("wk2", (INA, H * DH), f32)]
    wva = dram_in("wva", (INA, 2 * H * VG), f32)
    s_in = [[dram_in("s1a", (RPC, SLEN), i16), dram_in("s1b", (RPC, SLEN), i16)],
            [dram_in("s2a", (RPC, SLEN), i16), dram_in("s2b", (RPC, SLEN), i16)]]
    c_in = [dram_in("c1", (RPC, N), i16), dram_in("c2", (RPC, N), i16)]
    y = nc.dram_tensor("y", [RPC, 2 * H * DV], f32, kind="ExternalOutput").ap()

    HPD = cfg.HPT * DH
    with TC(nc) as tc:
        import contextlib
        with contextlib.ExitStack() as ctx:
            const_p = ctx.enter_context(tc.tile_pool(name="const", bufs=1))

            identf = const_p.tile([P, P], f32)
            masks.make_identity(nc, identf[:])
            identh = const_p.tile([P, P], f16)
            nc.vector.tensor_copy(identh[:], identf[:])
            expbias = const_p.tile([P, 1], f32)
            nc.gpsimd.memset(expbias[:], -1.5)

            # persistent projection outputs (fp16)
            kt = [[const_p.tile([HPD, N], f16, tag=f"kt{b}{hp}", name=f"kt{b}{hp}")
                   for hp in range(cfg.NHP)] for b in range(2)]
            qt = [[const_p.tile([HPD, RPC], f16, tag=f"qt{b}{hp}", name=f"qt{b}{hp}")
                   for hp in range(cfg.NHP)] for b in range(2)]
            VW = 2 * H * VG
            v_sb = const_p.tile([P, NJ * VW], f16)

            nkc = len(cfg.kchunks)
            # ---- projection phase (scoped pools, released afterwards) ----
            with tc.tile_pool(name="projsb", bufs=1) as proj_sb, \
                 tc.tile_pool(name="projps", bufs=2, space="PSUM") as proj_ps:
                xt, xq = [], []
                for o, csz in cfg.kchunks:
                    tf = proj_sb.tile([csz, N], f32, tag=f"xs{o}")
                    nc.sync.dma_start(tf[:], xta[o:o + csz, :])
                    tr = proj_sb.tile([csz, N], f32r, tag=f"xt{o}")
                    nc.vector.tensor_copy(tr[:], tf[:])
                    xt.append(tr)
                    tfq = proj_sb.tile([csz, RPC], f32, tag=f"xqs{o}")
                    nc.sync.dma_start(tfq[:], xtq[o:o + csz, :])
                    trq = proj_sb.tile([csz, RPC], f32r, tag=f"xq{o}")
                    nc.vector.tensor_copy(trq[:], tfq[:])
                    xq.append(trq)

                def load_w(ap, width, tag):
                    out = []
                    for o, csz in cfg.kchunks:
                        tf = proj_sb.tile([csz, width], f32, tag=f"{tag}s{o}")
                        nc.sync.dma_start(tf[:], ap[o:o + csz, :])
                        tr = proj_sb.tile([csz, width], f32r, tag=f"{tag}{o}")
                        nc.vector.tensor_copy(tr[:], tf[:])
                        out.append(tr)
                    return out

                wqt = [load_w(wq[b], H * DH, f"wq{b}") for b in range(2)]
                wkt = [load_w(wk[b], H * DH, f"wk{b}") for b in range(2)]
                wvt = load_w(wva, VW, "wv")

                for b in range(2):
                    for hp in range(cfg.NHP):
                        co = hp * HPD
                        for fc in range(0, N, cfg.FCH):
                            ps = proj_ps.tile([HPD, cfg.FCH], f32, tag="pk")
                            for kc in range(nkc):
                                nc.tensor.matmul(
                                    ps[:], wkt[b][kc][:, co:co + HPD],
                                    xt[kc][:, fc:fc + cfg.FCH],
                                    start=(kc == 0), stop=(kc == nkc - 1))
                            nc.scalar.copy(kt[b][hp][:, fc:fc + cfg.FCH], ps[:])
                        for fc in range(0, RPC, cfg.FCH):
                            fw = min(cfg.FCH, RPC - fc)
                            ps = proj_ps.tile([HPD, cfg.FCH], f32, tag="pq")
                            for kc in range(nkc):
                                nc.tensor.matmul(
                                    ps[:, 0:fw], wqt[b][kc][:, co:co + HPD],
                                    xq[kc][:, fc:fc + fw],
                                    start=(kc == 0), stop=(kc == nkc - 1))
                            nc.scalar.copy(qt[b][hp][:, fc:fc + fw], ps[:, 0:fw])
                for jc in range(NJ):
                    ps = proj_ps.tile([P, VW], f32, tag="pv")
                    for kc in range(nkc):
                        nc.tensor.matmul(
                            ps[:], xt[kc][:, jc * P:(jc + 1) * P], wvt[kc][:],
                            start=(kc == 0), stop=(kc == nkc - 1))
                    nc.scalar.copy(v_sb[:, jc * VW:(jc + 1) * VW], ps[:])

            # ---- main pools ----
            att_ps = ctx.enter_context(
                tc.tile_pool(name="att_ps", bufs=2, space="PSUM"))
            tp_ps = ctx.enter_context(
                tc.tile_pool(name="tp_ps", bufs=2, space="PSUM"))
            pv_ps = ctx.enter_context(
                tc.tile_pool(name="pv_ps", bufs=1, space="PSUM"))
            stream_p = ctx.enter_context(tc.tile_pool(name="stream", bufs=2))
            idx_p = ctx.enter_context(tc.tile_pool(name="idx", bufs=1))
            cidx_p = ctx.enter_context(tc.tile_pool(name="cidx", bufs=1))
            g_p = ctx.enter_context(tc.tile_pool(name="gdst", bufs=2))
            p_p = ctx.enter_context(tc.tile_pool(name="p", bufs=2))
            st_p = ctx.enter_context(tc.tile_pool(name="st", bufs=3))
            out_p = ctx.enter_context(tc.tile_pool(name="out", bufs=2))
            sm_p = ctx.enter_context(tc.tile_pool(name="sm", bufs=4))

            for rep in range(reps):
              for t in range(NT):
                rt0 = t * P
                sidx = [[idx_p.tile([P, SLEN], i16, tag=f"s{b}{hf}", name=f"sidx{b}{hf}_{t}_{rep}")
                         for hf in range(2)] for b in range(2)]
                cidx = [cidx_p.tile([P, N], i16, tag=f"c{b}", name=f"cidx{b}_{t}_{rep}") for b in range(2)]
                for b in range(2):
                    for hf in range(2):
                        nc.sync.dma_start(sidx[b][hf][:],
                                          s_in[b][hf][rt0:rt0 + P, :])
                    nc.sync.dma_start(cidx[b][:], c_in[b][rt0:rt0 + P, :])
                for h in range(H):
                    hp, ho = h // cfg.HPT, (h % cfg.HPT) * DH
                    streams = []
                    for b in range(2):
                        s = stream_p.tile([P, SLEN], f16, tag=f"stream{b}")
                        if "att" in skip:
                            nc.gpsimd.memset(s[:, 0:2], 1.0)
                        else:
                            for po in range(0, N, cfg.PIECE):
                                ps = att_ps.tile([P, cfg.PIECE], f32, tag="attps")
                                for fo in range(0, cfg.PIECE, cfg.FCH):
                                    nc.tensor.matmul(
                                        ps[:, fo:fo + cfg.FCH],
                                        qt[b][hp][ho:ho + DH, rt0:rt0 + P],
                                        kt[b][hp][ho:ho + DH,
                                                  po + fo:po + fo + cfg.FCH],
                                        start=True, stop=True)
                                nc.scalar.activation(s[:, po:po + cfg.PIECE],
                                                     ps[:], AF.Exp, bias=expbias[:])
                        streams.append(s)
                    gdst = []
                    for b in range(2):
                        src = streams[1 - b]
                        if "scatter" not in skip:
                            nc.gpsimd.local_scatter(
                                src[:, N:N + TW], src[:, 0:N], cidx[b][:],
                                channels=P, num_elems=TW, num_idxs=N)
                        if "scatter" not in skip:
                            for Lr, off in zip(cfg.passes[1:], cfg.passoff[1:]):
                                nc.vector.tensor_copy(src[:, off:off + Lr],
                                                      src[:, N:N + Lr])
                        halves = []
                        for hf in range(2):
                            gd = g_p.tile([P, HALF], f16, tag=f"gd{b}{hf}")
                            if "scatter" not in skip:
                                nc.gpsimd.local_scatter(
                                    gd[:], src[:, 0:SLEN], sidx[b][hf][:],
                                    channels=P, num_elems=HALF, num_idxs=SLEN)
                            else:
                                nc.gpsimd.memset(gd[:, 0:2], 1.0)
                            halves.append(gd)
                        gdst.append(halves)
                    for b in range(2):
                        pv = pv_ps.tile([VG, P], f32, tag="pv",
                                        name=f"pv{b}_{t}_{h}_{rep}")
                        if "pv" in skip:
                            nc.tensor.matmul(pv[:], v_sb[:, 0:VG], identh[:],
                                             start=True, stop=True)
                        else:
                            sf = p_p.tile([P, N], f16, tag="p",
                                          name=f"sf{b}_{t}_{h}_{rep}")
                            for hf in range(2):
                                nc.vector.tensor_mul(
                                    sf[:, hf * HALF:(hf + 1) * HALF],
                                    streams[b][:, hf * HALF:(hf + 1) * HALF],
                                    gdst[b][hf][:])
                            g_v = b * H + h
                            GRP = 8
                            for jg in range(0, NJ, GRP):
                                gn = min(GRP, NJ - jg)
                                tp = tp_ps.tile([P, GRP * P], f16, tag="tp",
                                                name=f"tp{b}_{t}_{h}_{jg}_{rep}")
                                for q in range(gn):
                                    nc.tensor.transpose(
                                        tp[:, q * P:(q + 1) * P],
                                        sf[:, (jg + q) * P:(jg + q + 1) * P],
                                        identh[:])
                                stt = st_p.tile([P, GRP * P], f16, tag="stt",
                                                name=f"stt{b}_{t}_{h}_{jg}_{rep}")
                                nc.vector.tensor_copy(stt[:, 0:gn * P],
                                                      tp[:, 0:gn * P])
                                for q in range(gn):
                                    jc = jg + q
                                    nc.tensor.matmul(
                                        pv[:], v_sb[:, jc * VW + g_v * VG:
                                                    jc * VW + (g_v + 1) * VG],
                                        stt[:, q * P:(q + 1) * P],
                                        start=(jc == 0), stop=(jc == NJ - 1))
                        pvs = out_p.tile([VG, P], f32, tag="pvs")
                        nc.vector.tensor_copy(pvs[:], pv[:])
                        ot = pv_ps.tile([P, VG], f32, tag="otp")
                        nc.tensor.transpose(ot[:], pvs[:], identf[0:VG, 0:VG])
                        rec = sm_p.tile([P, 1], f32, tag="rec")
                        nc.vector.reciprocal(rec[:], ot[:, DV:DV + 1])
                        res = out_p.tile([P, DV], f32, tag="res")
                        nc.vector.tensor_mul(res[:], ot[:, 0:DV],
                                             rec[:].broadcast_to([P, DV]))
                        nc.sync.dma_start(
                            y[rt0:rt0 + P, (b * H + h) * DV:
                              (b * H + h + 1) * DV], res[:])
    nc.compile()
    return nc


# ---------------------------------------------------------------------------
# entry point
# ---------------------------------------------------------------------------

_CACHE = {}


def _get_module(cfg):
    key = (cfg.N, cfg.IN, cfg.DH, cfg.DV, cfg.H, cfg.ncores)
    if key not in _CACHE:
        _CACHE[key] = build_module(cfg)
    return _CACHE[key]


def kernel(**inputs):
    """Full-input entry point: shards across 8 NeuronCores internally and
    returns the full (N, 2*H*DV) float32 output."""
    cfg = Cfg(N=3072, IN=256, DH=64, DV=32, H=4, ncores=8)
    nc = _get_module(cfg)
    maps = host_prep(cfg, inputs)
    for c, m in enumerate(maps):
        r0 = c * cfg.RPC
        m["xtq"] = np.ascontiguousarray(m["xta"][:, r0:r0 + cfg.RPC])
    res = run_bass_kernel_spmd(nc, maps, list(range(cfg.ncores)), trace=False)
    out = np.concatenate(
        [res.results[c]["y"] for c in range(cfg.ncores)], axis=0)
    return out.astype(np.float32)

